# revision 6
# baseline (speedup 1.0000x reference)
"""MoE expert-group kernel for 8 Trainium2 NeuronCores.

Strategy (expert-parallel, per the sharding hint):
  - Host computes the (tiny) router: logits = x @ Wg.T, top-2, softmax.
  - Tokens are gathered per expert on host ("dispatch"); core c owns
    experts (2c, 2c+1) and receives its two experts' tokens (transposed,
    zero-padded to a uniform capacity) plus its two experts' weights.
  - Each core runs a dense 2-layer MLP (relu(x@W1+b1)@W2+b2) over its
    gathered tokens in transposed layout: weights are the stationary
    matmul operand in their natural [in, out] layout, activations stream
    as the moving operand, biases become per-partition activation biases.
  - Host applies the per-(token, expert) softmax weight and scatter-adds
    ("combine") back to the full [8192, 1024] output, in the same expert
    order as the reference loop.

Only the dense MLP FLOPs (the compute-bound part, 1/8 of the dense-all-
experts reference) run on device; routing/gather/combine are O(N*E) or
O(N*D) host work.
"""

import os
import sys
import time

import numpy as np

sys.path.insert(0, "/opt/trn_rl_repo")

N_TOKENS = 8192
D_MODEL = 1024
D_HIDDEN = 2048
N_EXPERTS = 16
TOP_K = 2
N_CORES = 8
EPC = N_EXPERTS // N_CORES  # experts per core
KC1 = D_MODEL // 128   # k-chunks layer 1
MC1 = D_HIDDEN // 128  # m-chunks layer 1
KC2 = D_HIDDEN // 128  # k-chunks layer 2
MC2 = D_MODEL // 128   # m-chunks layer 2

# matmul dtype mode: "fp32" (bit-exact-ish, 1/4 rate), "fp32r" (full rate,
# reduced-precision multiplies), "bf16" (full rate, bf16 operands)
MM_MODE = os.environ.get("KERNEL_MM_MODE", "fp32r")


def _split_tiles(cap):
    """Split cap (multiple of 128) into moving-dim tiles <=512, all >=256
    when possible (fp32r runs full-rate only for moving dim >= 256)."""
    tiles = []
    r = cap
    while r > 512:
        take = 512 if (r - 512 == 0 or r - 512 >= 256) else 384
        tiles.append(take)
        r -= take
    tiles.append(r)
    return tiles


def build_program(cap, mode=MM_MODE, loop_reps=1):
    """Build the per-core program. loop_reps>1 wraps the body in a hardware
    For_i loop (identical work each iteration) for wall-clock timing."""
    import contextlib

    import concourse.mybir as mybir
    import concourse.tile as tile
    from concourse import bacc

    f32 = mybir.dt.float32
    # io_dt is the dtype of the matmul-operand tensors end-to-end (DRAM and
    # SBUF): the BIR verifier requires fp32r matmul inputs to be *produced*
    # as fp32r, so the whole operand chain carries the dtype.
    if mode == "fp32":
        io_dt = f32
    elif mode == "fp32r":
        io_dt = mybir.dt.float32r
    elif mode == "bf16":
        io_dt = mybir.dt.bfloat16
    else:
        raise ValueError(mode)

    tiles = _split_tiles(cap)

    nc = bacc.Bacc("TRN2", target_bir_lowering=False, debug=False)
    xt = nc.dram_tensor("xt", [EPC, D_MODEL, cap], io_dt, kind="ExternalInput").ap()
    w1 = nc.dram_tensor("w1", [EPC, D_MODEL, D_HIDDEN], io_dt, kind="ExternalInput").ap()
    b1 = nc.dram_tensor("b1", [EPC, D_HIDDEN], f32, kind="ExternalInput").ap()
    w2 = nc.dram_tensor("w2", [EPC, D_HIDDEN, D_MODEL], io_dt, kind="ExternalInput").ap()
    b2 = nc.dram_tensor("b2", [EPC, D_MODEL], f32, kind="ExternalInput").ap()
    yt = nc.dram_tensor("yt", [EPC, D_MODEL, cap], f32, kind="ExternalOutput").ap()

    Relu = mybir.ActivationFunctionType.Relu
    Ident = mybir.ActivationFunctionType.Identity

    with tile.TileContext(nc) as tc:
        with (
            tc.tile_pool(name="w1p", bufs=1) as w1p,
            tc.tile_pool(name="w2p", bufs=1) as w2p,
            tc.tile_pool(name="bp", bufs=2) as bp,
            tc.tile_pool(name="xp", bufs=1) as xp,
            tc.tile_pool(name="hp", bufs=1) as hp,
            tc.tile_pool(name="yp", bufs=4) as yp,
            tc.tile_pool(name="ps1", bufs=2, space="PSUM") as ps1,
            tc.tile_pool(name="ps2", bufs=2, space="PSUM") as ps2,
        ):
            loop_cm = (
                tc.For_i(0, loop_reps, 1)
                if loop_reps > 1
                else contextlib.nullcontext()
            )
            with loop_cm:
                for e in range(EPC):
                    w1t = w1p.tile([128, KC1, D_HIDDEN], io_dt, tag="w1t")
                    w1_src = w1[e].rearrange("(c p) m -> p c m", p=128)
                    for c in range(KC1):
                        nc.sync.dma_start(w1t[:, c, :], w1_src[:, c, :])
                    w2t = w2p.tile([128, KC2, D_MODEL], io_dt, tag="w2t")
                    w2_src = w2[e].rearrange("(c p) m -> p c m", p=128)
                    for c in range(KC2):
                        nc.sync.dma_start(w2t[:, c, :], w2_src[:, c, :])
                    b1t = bp.tile([128, MC1], f32, tag="b1t")
                    nc.sync.dma_start(b1t[:], b1[e].rearrange("(m p) -> p m", p=128))
                    b2t = bp.tile([128, MC2], f32, tag="b2t")
                    nc.sync.dma_start(b2t[:], b2[e].rearrange("(m p) -> p m", p=128))

                    xt_src = xt[e].rearrange("(c p) n -> p c n", p=128)
                    yt_dst = yt[e].rearrange("(c p) n -> p c n", p=128)

                    j0 = 0
                    for nt in tiles:
                        xtile = xp.tile([128, KC1, nt], io_dt, tag="xtile")
                        nc.sync.dma_start(xtile[:], xt_src[:, :, j0 : j0 + nt])

                        ht = hp.tile([128, KC2, nt], io_dt, tag="ht")
                        # layer 1: h.T = W1.T-free [m-chunk] accumulation
                        for m in range(MC1):
                            hps = ps1.tile([128, nt], f32, tag="hps")
                            for c in range(KC1):
                                nc.tensor.matmul(
                                    hps[:],
                                    lhsT=w1t[:, c, m * 128 : (m + 1) * 128],
                                    rhs=xtile[:, c, :],
                                    start=(c == 0),
                                    stop=(c == KC1 - 1),
                                )
                            nc.scalar.activation(
                                ht[:, m, :], hps[:], Relu, bias=b1t[:, m : m + 1]
                            )
                        # layer 2
                        for m in range(MC2):
                            yps = ps2.tile([128, nt], f32, tag="yps")
                            for c in range(KC2):
                                nc.tensor.matmul(
                                    yps[:],
                                    lhsT=w2t[:, c, m * 128 : (m + 1) * 128],
                                    rhs=ht[:, c, :],
                                    start=(c == 0),
                                    stop=(c == KC2 - 1),
                                )
                            ysb = yp.tile([128, nt], f32, tag="ysb")
                            nc.scalar.activation(
                                ysb[:], yps[:], Ident, bias=b2t[:, m : m + 1]
                            )
                            nc.sync.dma_start(yt_dst[:, m, j0 : j0 + nt], ysb[:])
                        j0 += nt
    nc.compile()
    return nc


def route(x, Wg):
    """Host router identical (up to fp rounding far below the top-2/3
    logit gap) to the reference: top-2 by logit, softmax over the pair."""
    logits = x.astype(np.float32, copy=False) @ Wg.astype(np.float32, copy=False).T
    n = logits.shape[0]
    rows = np.arange(n)
    i1 = np.argmax(logits, axis=1)
    v1 = logits[rows, i1]
    masked = logits.copy()
    masked[rows, i1] = -np.inf
    i2 = np.argmax(masked, axis=1)
    v2 = masked[rows, i2]
    d = np.exp((v2 - v1).astype(np.float64))
    wt1 = (1.0 / (1.0 + d)).astype(np.float32)
    wt2 = (d / (1.0 + d)).astype(np.float32)
    return i1, i2, wt1, wt2


def kernel(x, Wg, W1, b1, W2, b2):
    from concourse.bass_utils import run_bass_kernel_spmd

    x = np.ascontiguousarray(np.asarray(x, dtype=np.float32))
    Wg = np.asarray(Wg, dtype=np.float32)
    W1 = np.asarray(W1, dtype=np.float32)
    b1 = np.asarray(b1, dtype=np.float32)
    W2 = np.asarray(W2, dtype=np.float32)
    b2 = np.asarray(b2, dtype=np.float32)
    n_tokens = x.shape[0]

    i1, i2, wt1, wt2 = route(x, Wg)

    idxs, wts = [], []
    for e in range(N_EXPERTS):
        sel1 = i1 == e
        sel2 = i2 == e
        idx = np.concatenate([np.nonzero(sel1)[0], np.nonzero(sel2)[0]])
        w = np.concatenate([wt1[sel1], wt2[sel2]])
        idxs.append(idx)
        wts.append(w)

    max_count = max(len(i) for i in idxs)
    cap = max(256, -(-max_count // 128) * 128)

    io_np = np.float32 if MM_MODE != "bf16" else None
    if io_np is None:
        import ml_dtypes

        io_np = ml_dtypes.bfloat16

    in_maps = []
    for core in range(N_CORES):
        xt = np.zeros((EPC, D_MODEL, cap), dtype=io_np)
        for s in range(EPC):
            e = core * EPC + s
            xt[s, :, : len(idxs[e])] = x[idxs[e]].T.astype(io_np)
        in_maps.append(
            {
                "xt": xt,
                "w1": np.ascontiguousarray(W1[core * EPC : (core + 1) * EPC]).astype(io_np),
                "b1": np.ascontiguousarray(b1[core * EPC : (core + 1) * EPC]),
                "w2": np.ascontiguousarray(W2[core * EPC : (core + 1) * EPC]).astype(io_np),
                "b2": np.ascontiguousarray(b2[core * EPC : (core + 1) * EPC]),
            }
        )

    nc = build_program(cap)
    res = run_bass_kernel_spmd(nc, in_maps, core_ids=list(range(N_CORES)))

    out = np.zeros((n_tokens, D_MODEL), dtype=np.float32)
    for e in range(N_EXPERTS):
        core, s = e // EPC, e % EPC
        n_e = len(idxs[e])
        if n_e == 0:
            continue
        y = res.results[core]["yt"][s, :, :n_e].T  # [n_e, D]
        out[idxs[e]] += wts[e][:, None] * y
    return out


if __name__ == "__main__":
    rng = np.random.default_rng(0)
    x = rng.standard_normal((N_TOKENS, D_MODEL), dtype=np.float32)
    s_in = 1.0 / np.sqrt(D_MODEL)
    s_hid = 1.0 / np.sqrt(D_HIDDEN)
    Wg = rng.uniform(-s_in, s_in, (N_EXPERTS, D_MODEL)).astype(np.float32)
    W1 = rng.uniform(-s_in, s_in, (N_EXPERTS, D_MODEL, D_HIDDEN)).astype(np.float32)
    b1 = rng.uniform(-s_in, s_in, (N_EXPERTS, D_HIDDEN)).astype(np.float32)
    W2 = rng.uniform(-s_hid, s_hid, (N_EXPERTS, D_HIDDEN, D_MODEL)).astype(np.float32)
    b2 = rng.uniform(-s_hid, s_hid, (N_EXPERTS, D_MODEL)).astype(np.float32)
    t0 = time.time()
    out = kernel(x=x, Wg=Wg, W1=W1, b1=b1, W2=W2, b2=b2)
    print("kernel() wall:", time.time() - t0, "out", out.shape, out.dtype)
